# revision 1
# baseline (speedup 1.0000x reference)
"""CapsNet (semantic capsules + dynamic routing) on 8 TRN2 NeuronCores.

Sharding: sequence-shard the fc1/squash stage (each core owns 32 of 256
sequence positions = 256 of 2048 contraction elements), compute partial
priors for ALL capsules over the local contraction shard, ReduceScatter-add
so core i ends up with the full priors of capsule i, then do dynamic
routing for capsule i and emit output batches 8i..8i+8 (the reference's
flat reinterpret of vote maps capsule i exactly onto those batches).

HBM per core: ~6.3MB x-shard + ~12.6MB route_weights (active routes only;
softmax-masked routes contribute exactly 0) + 6.3MB output.
"""
import sys
from contextlib import ExitStack

if '/opt/trn_rl_repo' not in sys.path:
    sys.path.insert(0, '/opt/trn_rl_repo')

import numpy as np

import concourse.bass as bass
import concourse.bacc as bacc
import concourse.tile as tile
from concourse import mybir
import concourse.bass_utils as bass_utils

F32 = mybir.dt.float32
F32R = mybir.dt.float32r
AX = mybir.AxisListType
ALU = mybir.AluOpType
ACTF = mybir.ActivationFunctionType

N_CORES = 8
B, S, D = 64, 256, 768
CAP, NT = 8, 10
NCOL = NT * CAP          # 80 fc1 output cols (n*8+c)
SL = S // N_CORES        # 32 sequence positions per core
KL = SL * CAP            # 256 local contraction elements
L = S                    # 256 class dim
BLOC = B // N_CORES      # 8 output batches per core

_cache = {}


def _build(R: int, debug_mode=0):
    """Build + compile the SPMD program for R active routes.

    debug_mode: 0 normal; 1 skip collective (exec test); 2 stop after
    stage A; 3 stop after stage B; 4 stop after routing.
    """
    nc = bacc.Bacc("TRN2", target_bir_lowering=False, debug=False,
                   num_devices=N_CORES)

    xt = nc.dram_tensor("xt", [D, SL * B], F32, kind="ExternalInput")
    fw = nc.dram_tensor("fw", [128, 6 * NCOL], F32, kind="ExternalInput")
    fb = nc.dram_tensor("fb", [NCOL, 1], F32, kind="ExternalInput")
    rw = nc.dram_tensor("rw", [R * 2 * 4, 128, 2 * L], F32, kind="ExternalInput")
    lwt = nc.dram_tensor("lwt", [CAP, D], F32R, kind="ExternalInput")
    ident = nc.dram_tensor("ident", [128, 128], F32, kind="ExternalInput")
    out = nc.dram_tensor("out", [BLOC * S, D], F32, kind="ExternalOutput")

    NQ = (R + 1) // 2
    pairs = [list(range(2 * q, min(2 * q + 2, R))) for q in range(NQ)]
    ppart_q = [nc.dram_tensor(f"ppart{q}", [CAP, len(pairs[q]), B, L], F32)
               for q in range(NQ)]
    rsout_q = [nc.dram_tensor(f"rsout{q}", [len(pairs[q]), B, L], F32)
               for q in range(NQ)]
    ppart = nc.dram_tensor("ppart", [CAP, R, B, L], F32)
    wrmin = nc.dram_tensor("wrmin", [1, 4], F32)
    wrmout = nc.dram_tensor("wrmout", [8, 4], F32, addr_space="Shared")
    votedram = nc.dram_tensor("votedram", [B, L], F32R)
    rsout = nc.dram_tensor("rsout", [R, B, L], F32)

    ecnt = [0]

    def copy_alt(dst, src):
        """Alternate PSUM->SBUF copies between ACT and DVE."""
        ecnt[0] += 1
        if ecnt[0] % 2 == 0:
            nc.scalar.copy(dst, src)
        else:
            nc.vector.tensor_copy(dst, src)

    with tile.TileContext(nc) as tc:
        with (
            tc.tile_pool(name="const", bufs=1) as constp,
            tc.tile_pool(name="xtp", bufs=6) as xtp,
            tc.tile_pool(name="stageA", bufs=1) as sa,
            tc.tile_pool(name="junk", bufs=2) as junkp,
            tc.tile_pool(name="rwp", bufs=24) as rwp,
            tc.tile_pool(name="ppsb", bufs=10) as ppsbp,
            tc.tile_pool(name="route", bufs=1) as rt,
            tc.tile_pool(name="acc", bufs=2) as accp,
            tc.tile_pool(name="osb", bufs=4) as osbp,
        ):
            # ---- constants in ----
            fw_sb = constp.tile([128, 6 * NCOL], F32, tag="fw")
            nc.sync.dma_start(out=fw_sb[:], in_=fw[:])
            fb_sb = constp.tile([NCOL, 1], F32, tag="fb")
            nc.sync.dma_start(out=fb_sb[:], in_=fb[:])
            lwt_sb = constp.tile([CAP, D], F32R, tag="lwt")
            nc.sync.dma_start(out=lwt_sb[:], in_=lwt[:])
            id_sb = constp.tile([128, 128], F32, tag="ident")
            nc.sync.dma_start(out=id_sb[:], in_=ident[:])

            # ================= stage A: fc1 -> uT ======================
            ps_stack = ExitStack()
            psA = ps_stack.enter_context(
                tc.tile_pool(name="psA", bufs=1, space="PSUM"))
            psT = ps_stack.enter_context(
                tc.tile_pool(name="psT", bufs=3, space="PSUM"))
            xt_t = []
            for j in range(6):
                t = xtp.tile([128, SL * B], F32, tag="xt")
                nc.sync.dma_start(out=t[:], in_=xt[j * 128:(j + 1) * 128, :])
                xt_t.append(t)

            psum_sem = psA.tile([NCOL, SL * B], F32, tag="sem")
            for j in range(6):
                for n4 in range(4):
                    nc.tensor.matmul(
                        psum_sem[:, n4 * 512:(n4 + 1) * 512],
                        lhsT=fw_sb[:, j * NCOL:(j + 1) * NCOL],
                        rhs=xt_t[j][:, n4 * 512:(n4 + 1) * 512],
                        start=(j == 0), stop=(j == 5),
                    )
            semT_sb = sa.tile([NCOL, SL * B], F32, tag="semT")
            # evacuate PSUM + add fc1 bias (per-partition scalar)
            nc.vector.tensor_scalar_add(semT_sb[:], psum_sem[:], fb_sb[0:NCOL, 0:1])

            # per-s transpose: semT [80, 64] -> u_all [64(b), s*80+nc]
            u_all = sa.tile([B, SL * NCOL], F32, tag="u_all")
            for s in range(SL):
                ps_t = psT.tile([B, NCOL], F32, tag="pst")
                nc.tensor.transpose(
                    ps_t[:], semT_sb[:, s * B:(s + 1) * B], id_sb[0:NCOL, 0:NCOL])
                copy_alt(u_all[:, s * NCOL:(s + 1) * NCOL], ps_t[:])

            # squash over n (free-strided)
            tmp2 = sa.tile([B, SL * NCOL], F32, tag="tmp2")
            nc.vector.tensor_mul(tmp2[:], u_all[:], u_all[:])
            sq = sa.tile([B, SL * CAP], F32, tag="sq")
            nc.vector.tensor_reduce(
                out=sq[:].rearrange("p (s c) -> p s c", c=CAP),
                in_=tmp2[:].rearrange("p (s n c) -> p s c n", n=NT, c=CAP),
                axis=AX.X, op=ALU.add,
            )
            s1 = sa.tile([B, SL * CAP], F32, tag="s1")
            nc.scalar.activation(s1[:], sq[:], ACTF.Sqrt)
            s2 = sa.tile([B, SL * CAP], F32, tag="s2")
            nc.vector.tensor_scalar_add(s2[:], sq[:], 1.0)
            s3 = sa.tile([B, SL * CAP], F32, tag="s3")
            nc.vector.reciprocal(s3[:], s2[:])
            scl = sa.tile([B, SL * CAP], F32, tag="scl")
            nc.vector.tensor_mul(scl[:], s1[:], s3[:])
            # expand scale over the R active routes, r-major:
            # scl6[b, r*256 + s*8 + c] = scl[b, s*8+c]
            scl6 = sa.tile([B, R * SL * CAP], F32, tag="scl6")
            for r in range(R):
                nc.vector.tensor_copy(
                    scl6[:, r * KL:(r + 1) * KL], scl[:])
            # u_act[b, r*256 + s*8 + c] = u_all[b, s*80 + r*8 + c] * scl
            u_act = sa.tile([B, R * SL * CAP], F32, tag="u_act")
            nc.vector.tensor_mul(
                u_act[:],
                u_all[:].rearrange("p (s n c) -> p n s c", n=NT, c=CAP)[:, 0:R, :, :],
                scl6[:],
            )

            # uT tiles [128(k=s*8+c), 64(b)] per (r, half)
            uT_sb = []
            for h in range(2):
                uTh = sa.tile([128, R * B], F32, tag=f"uT{h}", name=f"uT{h}")
                uT_sb.append(uTh)
            for r in range(R):
                for h in range(2):
                    psU = psT.tile([128, B], F32, tag="pst")
                    nc.tensor.transpose(
                        psU[:],
                        u_act[:, r * KL + h * 128:r * KL + (h + 1) * 128],
                        id_sb[0:B, 0:B],
                    )
                    copy_alt(uT_sb[h][:, r * B:(r + 1) * B], psU[:])
            ps_stack.close()

            if debug_mode == 2:
                nc.sync.dma_start(out=out[0:B, 0:D], in_=u_act[:, 0:D])

            # ================= stage B: partial priors =================
            if debug_mode != 2:
                ps_stack = ExitStack()
                psPP = ps_stack.enter_context(
                    tc.tile_pool(name="psPP", bufs=6, space="PSUM"))
                for q in range(NQ):
                    for ri, r in enumerate(pairs[q]):
                        for cp in range(4):
                            rwt = []
                            for kt in range(2):
                                t = rwp.tile([128, 2 * L], F32, tag="rw")
                                nc.sync.dma_start(
                                    out=t[:], in_=rw[(r * 2 + kt) * 4 + cp])
                                rwt.append(t)
                            pspp = psPP.tile([B, 2 * L], F32, tag="pp")
                            for kt in range(2):
                                nc.tensor.matmul(
                                    pspp[:],
                                    lhsT=uT_sb[kt][:, r * B:(r + 1) * B],
                                    rhs=rwt[kt][:],
                                    start=(kt == 0), stop=(kt == 1),
                                )
                            pp_sb = ppsbp.tile([B, 2 * L], F32, tag="ppsb")
                            copy_alt(pp_sb[:], pspp[:])
                            nc.sync.dma_start(out=ppart_q[q][2 * cp, ri],
                                              in_=pp_sb[:, 0:L])
                            nc.sync.dma_start(out=ppart_q[q][2 * cp + 1, ri],
                                              in_=pp_sb[:, L:2 * L])
                    # chunked ReduceScatter overlaps later chunks' matmuls
                    if debug_mode != 1:
                        nc.gpsimd.collective_compute(
                            "ReduceScatter", ALU.add,
                            replica_groups=[list(range(N_CORES))],
                            ins=[ppart_q[q][:]], outs=[rsout_q[q][:]],
                        )
                ps_stack.close()

            if debug_mode == 3:
                nc.sync.dma_start(out=out[0:R * B, 0:L],
                                  in_=ppart[0].rearrange("r b l -> (r b) l"))

            if debug_mode in (0, 1, 4, 5, 6, 7):
                rlevel = {5: 0, 6: 1, 7: 2}.get(debug_mode, 9)
                # ============= stage C: dynamic routing ================
                pri = rt.tile([B, R * L], F32, tag="pri")
                for q in range(NQ):
                    for ri, r in enumerate(pairs[q]):
                        nc.sync.dma_start(out=pri[:, r * L:(r + 1) * L],
                                          in_=rsout_q[q][ri])

                def pri_r(r):
                    return pri[:, r * L:(r + 1) * L]

                if debug_mode == 5:
                    nc.sync.dma_start(out=out[0:B, 0:L], in_=pri[:, 0:L])
                if rlevel >= 1:
                    # iter 0: probs uniform over R active routes.
                    ssum = rt.tile([B, L], F32, tag="ssum")
                    if R == 1:
                        nc.vector.tensor_copy(ssum[:], pri_r(0))
                    else:
                        nc.vector.tensor_add(ssum[:], pri_r(0), pri_r(1))
                        for r in range(2, R):
                            nc.vector.tensor_add(ssum[:], ssum[:], pri_r(r))

                logits = rt.tile([B, R], F32, tag="logits")
                vote = rt.tile([B, L], F32, tag="vote")

                def squash_scale(v, sqscale, tag):
                    """[B,1] tile: sqrt(sq)/(1+sq), sq = sum(v*v)*sqscale."""
                    sqv = rt.tile([B, 1], F32, tag=tag + "sq", name=tag + "sq")
                    junk = junkp.tile([B, L], F32, tag="junk", name="junk")
                    sqr = rt.tile([B, 1], F32, tag=tag + "sr", name=tag + "sr")
                    nc.vector.scalar_tensor_tensor(
                        out=junk[:], in0=v, scalar=1.0, in1=v,
                        op0=ALU.mult, op1=ALU.mult, accum_out=sqr[:])
                    nc.vector.tensor_scalar_mul(sqv[:], sqr[:], float(sqscale))
                    a = rt.tile([B, 1], F32, tag=tag + "a", name=tag + "a")
                    nc.scalar.activation(a[:], sqv[:], ACTF.Sqrt)
                    bb = rt.tile([B, 1], F32, tag=tag + "b", name=tag + "b")
                    nc.vector.tensor_scalar_add(bb[:], sqv[:], 1.0)
                    cc = rt.tile([B, 1], F32, tag=tag + "c", name=tag + "c")
                    nc.vector.reciprocal(cc[:], bb[:])
                    sc = rt.tile([B, 1], F32, tag=tag + "s", name=tag + "s")
                    nc.vector.tensor_mul(sc[:], a[:], cc[:])
                    return sc

                def raw_delta(vsrc, dst):
                    """dst[b, r] = sum_l pri_r * vsrc."""
                    for r in range(R):
                        junk = junkp.tile([B, L], F32, tag="junk", name="junk")
                        nc.vector.scalar_tensor_tensor(
                            out=junk[:], in0=pri_r(r), scalar=1.0, in1=vsrc,
                            op0=ALU.mult, op1=ALU.mult,
                            accum_out=dst[:, r:r + 1])

                def softmax_and_vote(lg, vdst):
                    mx = rt.tile([B, 1], F32, tag="mx", name="mx")
                    nc.vector.tensor_reduce(out=mx[:], in_=lg[:], axis=AX.X,
                                            op=ALU.max)
                    ngm = rt.tile([B, 1], F32, tag="ngm", name="ngm")
                    nc.vector.tensor_scalar_mul(ngm[:], mx[:], -1.0)
                    ex = rt.tile([B, R], F32, tag="ex", name="ex")
                    nc.scalar.activation(ex[:], lg[:], ACTF.Exp,
                                         bias=ngm[0:B, 0:1])
                    se = rt.tile([B, 1], F32, tag="se", name="se")
                    nc.vector.tensor_reduce(out=se[:], in_=ex[:], axis=AX.X,
                                            op=ALU.add)
                    ri = rt.tile([B, 1], F32, tag="ri", name="ri")
                    nc.vector.reciprocal(ri[:], se[:])
                    pr = rt.tile([B, R], F32, tag="pr", name="pr")
                    nc.vector.tensor_scalar_mul(pr[:], ex[:], ri[0:B, 0:1])
                    # vote = sum_r probs_r * pri_r
                    acc = accp.tile([B, L], F32, tag="acc", name="acc")
                    nc.vector.tensor_scalar_mul(acc[:], pri_r(0), pr[0:B, 0:1])
                    for r in range(1, R):
                        acc2 = accp.tile([B, L], F32, tag="acc", name="acc")
                        nc.vector.scalar_tensor_tensor(
                            out=acc2[:], in0=pri_r(r), scalar=pr[0:B, r:r + 1],
                            in1=acc[:], op0=ALU.mult, op1=ALU.add)
                        acc = acc2
                    nc.vector.tensor_copy(vdst, acc[:])

                if rlevel >= 1:
                    # iter 0
                    sc0 = squash_scale(ssum[:], 1.0 / (R * R), "i0")
                    rd0 = rt.tile([B, R], F32, tag="rd0")
                    raw_delta(ssum[:], rd0)
                    t0 = rt.tile([B, R], F32, tag="t0")
                    nc.vector.tensor_scalar_mul(t0[:], rd0[:], sc0[0:B, 0:1])
                    nc.vector.tensor_scalar_mul(logits[:], t0[:], 1.0 / R)

                if debug_mode == 6:
                    nc.sync.dma_start(out=out[0:B, 0:R], in_=logits[:])
                if rlevel >= 2:
                    # iter 1
                    softmax_and_vote(logits, vote[:])
                    sc1 = squash_scale(vote[:], 1.0, "i1")
                    rd1 = rt.tile([B, R], F32, tag="rd1")
                    raw_delta(vote[:], rd1)
                    t1 = rt.tile([B, R], F32, tag="t1")
                    nc.vector.tensor_scalar_mul(t1[:], rd1[:], sc1[0:B, 0:1])
                    lg2 = rt.tile([B, R], F32, tag="lg2")
                    nc.vector.tensor_add(lg2[:], logits[:], t1[:])

                if debug_mode == 7:
                    nc.sync.dma_start(out=out[0:B, 0:L], in_=vote[:])
                if rlevel >= 3:
                    # iter 2 (final vote; reference uses the un-squashed vote)
                    softmax_and_vote(lg2, vote[:])

            if debug_mode == 4:
                nc.sync.dma_start(out=out[0:B, 0:L], in_=vote[:])

            if debug_mode in (0, 1):
                # ============= stage D: reinterpret + final matmul =====
                # via DRAM: hT[c, m*256+j*32+sl] = vote[m*8+j, sl*8+c]
                vote_r = rt.tile([B, L], F32R, tag="vote_r")
                nc.vector.tensor_copy(vote_r[:], vote[:])
                nc.sync.dma_start(out=votedram[:], in_=vote_r[:])
                hT = rt.tile([CAP, BLOC * S], F32R, tag="hT")
                nc.sync.dma_start(
                    out=hT[:].rearrange("p (m j sl) -> p m j sl", m=BLOC, j=8),
                    in_=votedram[:].rearrange("(m j) (sl c) -> c m j sl",
                                              j=8, c=CAP),
                )

                ps_stack = ExitStack()
                psO = ps_stack.enter_context(
                    tc.tile_pool(name="psO", bufs=3, space="PSUM"))
                for t in range(16):
                    pso = psO.tile([128, D], F32, tag="pso")
                    nc.tensor.matmul(
                        pso[:, 0:512], lhsT=hT[:, t * 128:(t + 1) * 128],
                        rhs=lwt_sb[:, 0:512], start=True, stop=True)
                    nc.tensor.matmul(
                        pso[:, 512:D], lhsT=hT[:, t * 128:(t + 1) * 128],
                        rhs=lwt_sb[:, 512:D], start=True, stop=True)
                    o_sb = osbp.tile([128, D], F32, tag="osb")
                    copy_alt(o_sb[:], pso[:])
                    nc.sync.dma_start(out=out[t * 128:(t + 1) * 128, :],
                                      in_=o_sb[:])
                ps_stack.close()

    nc.compile()
    return nc


def _prep_inputs(x, task, fc1_w, fc1_b, route_weights, larger_w):
    R = int(task) + 1
    fw = np.ascontiguousarray(
        fc1_w.reshape(NCOL, D).T.reshape(6, 128, NCOL).transpose(1, 0, 2)
    ).reshape(128, 6 * NCOL).astype(np.float32)
    fb = np.ascontiguousarray(fc1_b.reshape(NCOL, 1)).astype(np.float32)
    lwt = np.ascontiguousarray(larger_w.T).astype(np.float32)
    ident = np.eye(128, dtype=np.float32)
    in_maps = []
    for i in range(N_CORES):
        xt_i = np.ascontiguousarray(
            x[:, i * SL:(i + 1) * SL, :].transpose(2, 1, 0)
        ).reshape(D, SL * B).astype(np.float32)
        rw_i = np.ascontiguousarray(
            route_weights[:, :R, i * KL:(i + 1) * KL, :]
            .reshape(4, 2, R, 2, 128, L)
            .transpose(2, 3, 0, 4, 1, 5)
        ).reshape(R * 2 * 4, 128, 2 * L).astype(np.float32)
        in_maps.append({"xt": xt_i, "fw": fw, "fb": fb, "rw": rw_i,
                        "lwt": lwt, "ident": ident})
    return in_maps


def kernel(x, task, fc1_w, fc1_b, route_weights, larger_w, larger_b,
           _return_results=False):
    x = np.asarray(x, dtype=np.float32)
    fc1_w = np.asarray(fc1_w, dtype=np.float32)
    fc1_b = np.asarray(fc1_b, dtype=np.float32)
    route_weights = np.asarray(route_weights, dtype=np.float32)
    larger_w = np.asarray(larger_w, dtype=np.float32)
    larger_b = np.asarray(larger_b, dtype=np.float32)
    R = int(task) + 1

    if R not in _cache:
        _cache[R] = _build(R)
    nc = _cache[R]

    in_maps = _prep_inputs(x, task, fc1_w, fc1_b, route_weights, larger_w)
    res = bass_utils.run_bass_kernel_spmd(nc, in_maps, list(range(N_CORES)))

    full = np.empty((B, S, D), dtype=np.float32)
    for i in range(N_CORES):
        full[i * BLOC:(i + 1) * BLOC] = res.results[i]["out"].reshape(BLOC, S, D)
    if np.any(larger_b):
        full = full + larger_b[None, None, :]
    if _return_results:
        return full, res
    return full



# revision 28
# speedup vs baseline: 1.3149x; 1.3149x over previous
"""CapsNet (semantic capsules + dynamic routing) on 8 TRN2 NeuronCores.

Sharding v4: capsule-parallel stage B; u exchanged as a bf16 hi/lo pair
via two pipelined AllGathers.

- Routing is numerically knife-edged (capsule 7 has near-tied routes), so
  priors need ~2^-16 effective precision: hardware float32r turned out to
  be ~fp16-grade (rel 0.045, fails), so stage B uses an explicit 2-term
  bf16 split: pri = uh*wh + uh*wl + ul*wh (lo*lo dropped), which models at
  rel 0.0056 vs the 2e-2 gate. Stage A (fc1 + squash) stays plain fp32.
- Stage A (sequence-sharded): each core owns 32 of 256 sequence positions
  -> local uT, split on-device into bf16 hi + lo tiles.
- Two AllGathers (hi, then lo; 196KB/rank in each). Phase-1 matmuls
  (uh*wh + uh*wl, 2/3 of the work) run under the lo AllGather; phase 2
  (ul*wh) finishes after it.
- rw is pre-split on the host into bf16 hi/lo [R, 128, KT*L] tensors.
  wl tiles share a tile tag with the xt tiles so their DMAs queue behind
  stage A's critical x loads instead of competing for HBM.
- Stage C: dynamic routing (3 iters) for capsule i, f32 vector ops.
- Stage D: the reference's flat reinterpret maps capsule i exactly onto
  output batches 8i..8i+8 (bf16 DRAM round trip + bf16 matmuls).
"""
import sys
from contextlib import ExitStack

if '/opt/trn_rl_repo' not in sys.path:
    sys.path.insert(0, '/opt/trn_rl_repo')

import numpy as np
import ml_dtypes

import concourse.bass as bass
import concourse.bacc as bacc
import concourse.tile as tile
from concourse import mybir
import concourse.bass_utils as bass_utils

F32 = mybir.dt.float32
BF16 = mybir.dt.bfloat16
AX = mybir.AxisListType
ALU = mybir.AluOpType
ACTF = mybir.ActivationFunctionType

N_CORES = 8
B, S, D = 64, 256, 768
CAP, NT = 8, 10
NCOL = NT * CAP          # 80 fc1 output cols (n*8+c)
SL = S // N_CORES        # 32 sequence positions per core
KL = SL * CAP            # 256 local contraction elements
KT = (S * CAP) // 128    # 16 k-tiles of 128 in the full contraction
L = S                    # 256 class dim
BLOC = B // N_CORES      # 8 output batches per core

_cache = {}


def _build(R: int, debug_taps=0):
    nc = bacc.Bacc("TRN2", target_bir_lowering=False, debug=False,
                   num_devices=N_CORES)

    RB = R * B
    xt = nc.dram_tensor("xt", [D, SL * B], F32, kind="ExternalInput")
    fw = nc.dram_tensor("fw", [128, 6 * NCOL], F32, kind="ExternalInput")
    fb = nc.dram_tensor("fb", [NCOL, 1], F32, kind="ExternalInput")
    rwh = nc.dram_tensor("rwh", [R, 128, KT * L], BF16, kind="ExternalInput")
    rwl = nc.dram_tensor("rwl", [R, 128, KT * L], BF16, kind="ExternalInput")
    lwt = nc.dram_tensor("lwt", [CAP, D], BF16, kind="ExternalInput")
    ident = nc.dram_tensor("ident", [128, 128], F32, kind="ExternalInput")
    out = nc.dram_tensor("out", [BLOC * S, D], BF16, kind="ExternalOutput")

    uloc_h = nc.dram_tensor("uloc_h", [2, 128, RB], BF16)
    uloc_l = nc.dram_tensor("uloc_l", [2, 128, RB], BF16)
    ufull_h = nc.dram_tensor("ufull_h", [KT, 128, RB], BF16,
                             addr_space="Shared")
    ufull_l = nc.dram_tensor("ufull_l", [KT, 128, RB], BF16,
                             addr_space="Shared")
    if debug_taps:
        dbg_pri = nc.dram_tensor("dbg_pri", [B, R * L], F32,
                                 kind="ExternalOutput")

    ecnt = [0]

    def copy_alt(dst, src):
        """Alternate PSUM->SBUF copies between ACT and DVE."""
        ecnt[0] += 1
        if ecnt[0] % 2 == 0:
            nc.scalar.copy(dst, src)
        else:
            nc.vector.tensor_copy(dst, src)

    with tile.TileContext(nc) as tc:
        with (
            tc.tile_pool(name="const", bufs=1) as constp,
            tc.tile_pool(name="persist", bufs=1) as pp,
            tc.tile_pool(name="stream", bufs=6) as streamp,
            tc.tile_pool(name="junk", bufs=2) as junkp,
            tc.tile_pool(name="route", bufs=1) as rt,
            tc.tile_pool(name="acc", bufs=2) as accp,
            tc.tile_pool(name="osb", bufs=4) as osbp,
            tc.tile_pool(name="vdram", bufs=1, space="DRAM") as vdp,
        ):
            # ---- constants in ----
            fw_sb = constp.tile([128, 6 * NCOL], F32, tag="fw")
            nc.sync.dma_start(out=fw_sb[:], in_=fw[:])
            fb_sb = constp.tile([NCOL, 1], F32, tag="fb")
            nc.sync.dma_start(out=fb_sb[:], in_=fb[:])
            lwt_sb = constp.tile([CAP, D], BF16, tag="lwt")
            nc.sync.dma_start(out=lwt_sb[:], in_=lwt[:])
            id_sb = constp.tile([128, 128], F32, tag="ident")
            nc.sync.dma_start(out=id_sb[:], in_=ident[:])

            # ================= stage A: fc1 -> local uT ================
            a_stack = ExitStack()
            sa = a_stack.enter_context(tc.tile_pool(name="stageA", bufs=1))
            psA = a_stack.enter_context(
                tc.tile_pool(name="psA", bufs=1, space="PSUM"))
            psT = a_stack.enter_context(
                tc.tile_pool(name="psT", bufs=3, space="PSUM"))
            # xt tiles share tag "xtwl" with the (later) wl tiles: the wl
            # DMAs inherit a WAR dep on stage A's reads, so they cannot
            # crowd the critical x loads out of HBM bandwidth.
            xt_t = []
            for j in range(6):
                t = streamp.tile([128, SL * B], F32, tag="xtwl")
                nc.sync.dma_start(out=t[:], in_=xt[j * 128:(j + 1) * 128, :])
                xt_t.append(t)

            psum_sem = psA.tile([NCOL, SL * B], F32, tag="sem")
            for j in range(6):
                for n4 in range(4):
                    nc.tensor.matmul(
                        psum_sem[:, n4 * 512:(n4 + 1) * 512],
                        lhsT=fw_sb[:, j * NCOL:(j + 1) * NCOL],
                        rhs=xt_t[j][:, n4 * 512:(n4 + 1) * 512],
                        start=(j == 0), stop=(j == 5),
                    )
            semT_sb = sa.tile([NCOL, SL * B], F32, tag="semT")
            # evacuate PSUM + add fc1 bias (per-partition scalar)
            nc.vector.tensor_scalar_add(semT_sb[:], psum_sem[:], fb_sb[0:NCOL, 0:1])

            # per-s transpose: semT [80, 64] -> u_all [64(b), s*80+nc]
            u_all = sa.tile([B, SL * NCOL], F32, tag="u_all")
            for s in range(SL):
                ps_t = psT.tile([B, NCOL], F32, tag="pst")
                nc.tensor.transpose(
                    ps_t[:], semT_sb[:, s * B:(s + 1) * B], id_sb[0:NCOL, 0:NCOL])
                copy_alt(u_all[:, s * NCOL:(s + 1) * NCOL], ps_t[:])

            # squash over n (free-strided)
            tmp2 = sa.tile([B, SL * NCOL], F32, tag="tmp2")
            nc.vector.tensor_mul(tmp2[:], u_all[:], u_all[:])
            sq = sa.tile([B, SL * CAP], F32, tag="sq")
            nc.vector.tensor_reduce(
                out=sq[:].rearrange("p (s c) -> p s c", c=CAP),
                in_=tmp2[:].rearrange("p (s n c) -> p s c n", n=NT, c=CAP),
                axis=AX.X, op=ALU.add,
            )
            s1 = sa.tile([B, SL * CAP], F32, tag="s1")
            nc.scalar.activation(s1[:], sq[:], ACTF.Sqrt)
            s2 = sa.tile([B, SL * CAP], F32, tag="s2")
            nc.vector.tensor_scalar_add(s2[:], sq[:], 1.0)
            s3 = sa.tile([B, SL * CAP], F32, tag="s3")
            nc.vector.reciprocal(s3[:], s2[:])
            scl = sa.tile([B, SL * CAP], F32, tag="scl")
            nc.vector.tensor_mul(scl[:], s1[:], s3[:])
            # expand scale over the R active routes, r-major:
            # scl6[b, r*256 + s*8 + c] = scl[b, s*8+c]
            scl6 = sa.tile([B, R * SL * CAP], F32, tag="scl6")
            for r in range(R):
                nc.vector.tensor_copy(
                    scl6[:, r * KL:(r + 1) * KL], scl[:])
            # u_act[b, r*256 + s*8 + c] = u_all[b, s*80 + r*8 + c] * scl
            u_act = sa.tile([B, R * SL * CAP], F32, tag="u_act")
            nc.vector.tensor_mul(
                u_act[:],
                u_all[:].rearrange("p (s n c) -> p n s c", n=NT, c=CAP)[:, 0:R, :, :],
                scl6[:],
            )

            # local uT tiles [128(k_local), R*64(b)], split bf16 hi/lo
            uTh_sb = [pp.tile([128, RB], BF16, tag=f"uTh{h}", name=f"uTh{h}")
                      for h in range(2)]
            uTl_sb = [pp.tile([128, RB], BF16, tag=f"uTl{h}", name=f"uTl{h}")
                      for h in range(2)]
            for r in range(R):
                for h in range(2):
                    psU = psT.tile([128, B], F32, tag="pst")
                    nc.tensor.transpose(
                        psU[:],
                        u_act[:, r * KL + h * 128:r * KL + (h + 1) * 128],
                        id_sb[0:B, 0:B],
                    )
                    hi = uTh_sb[h][:, r * B:(r + 1) * B]
                    lo = uTl_sb[h][:, r * B:(r + 1) * B]
                    copy_alt(hi, psU[:])
                    # lo = f32(psU) - bf16(hi), rounded to bf16
                    nc.vector.tensor_sub(lo, psU[:], hi)
            a_stack.close()

            for h in range(2):
                nc.sync.dma_start(out=uloc_h[h], in_=uTh_sb[h][:])
            for h in range(2):
                nc.sync.dma_start(out=uloc_l[h], in_=uTl_sb[h][:])

            # ============== AllGather u (hi, then lo) ==================
            nc.gpsimd.collective_compute(
                "AllGather", ALU.bypass,
                replica_groups=[list(range(N_CORES))],
                ins=[uloc_h[:]], outs=[ufull_h[:]],
            )
            nc.gpsimd.collective_compute(
                "AllGather", ALU.bypass,
                replica_groups=[list(range(N_CORES))],
                ins=[uloc_l[:]], outs=[ufull_l[:]],
            )

            # rw hi streams in while stage A + the hi collective run
            rwh_sb = []
            for r in range(R):
                t = streamp.tile([128, KT * L], BF16, tag="rwh")
                nc.sync.dma_start(out=t[:], in_=rwh[r])
                rwh_sb.append(t)
            # wl tiles reuse the xt slots (tag "xtwl") -> queued after A
            rwl_sb = []
            for r in range(R):
                t = streamp.tile([128, KT * L], BF16, tag="xtwl")
                nc.sync.dma_start(out=t[:], in_=rwl[r])
                rwl_sb.append(t)

            uTfh = pp.tile([128, KT * RB], BF16, tag="uTfh")
            for t in range(KT):
                nc.sync.dma_start(out=uTfh[:, t * RB:(t + 1) * RB],
                                  in_=ufull_h[t])
            uTfl = pp.tile([128, KT * RB], BF16, tag="uTfl")
            for t in range(KT):
                nc.sync.dma_start(out=uTfl[:, t * RB:(t + 1) * RB],
                                  in_=ufull_l[t])

            # ========= stage B: priors for this core's capsule =========
            # pri = uh*wh + uh*wl + ul*wh accumulated per-route in PSUM.
            # Phase 1 (uh terms) only needs the hi AllGather; phase 2
            # (ul*wh) runs after the lo AllGather lands.
            ps_stack = ExitStack()
            psB = ps_stack.enter_context(
                tc.tile_pool(name="psB", bufs=1, space="PSUM"))
            pri = rt.tile([B, R * L], F32, tag="pri")
            pspri = []
            for r in range(R):
                ps = psB.tile([B, L], F32, tag=f"pri{r}", name=f"pri{r}")
                pspri.append(ps)
                for kt in range(KT):
                    nc.tensor.matmul(
                        ps[:],
                        lhsT=uTfh[:, kt * RB + r * B:kt * RB + (r + 1) * B],
                        rhs=rwh_sb[r][:, kt * L:(kt + 1) * L],
                        start=(kt == 0), stop=False,
                    )
                for kt in range(KT):
                    nc.tensor.matmul(
                        ps[:],
                        lhsT=uTfh[:, kt * RB + r * B:kt * RB + (r + 1) * B],
                        rhs=rwl_sb[r][:, kt * L:(kt + 1) * L],
                        start=False, stop=False,
                    )
            for r in range(R):
                for kt in range(KT):
                    nc.tensor.matmul(
                        pspri[r][:],
                        lhsT=uTfl[:, kt * RB + r * B:kt * RB + (r + 1) * B],
                        rhs=rwh_sb[r][:, kt * L:(kt + 1) * L],
                        start=False, stop=(kt == KT - 1),
                    )
                copy_alt(pri[:, r * L:(r + 1) * L], pspri[r][:])
            ps_stack.close()

            if debug_taps:
                nc.sync.dma_start(out=dbg_pri[:], in_=pri[:])

            def pri_r(r):
                return pri[:, r * L:(r + 1) * L]

            # ============= stage C: dynamic routing ====================
            # iter 0: probs uniform over R active routes.
            ssum = rt.tile([B, L], F32, tag="ssum")
            if R == 1:
                nc.vector.tensor_copy(ssum[:], pri_r(0))
            else:
                nc.vector.tensor_add(ssum[:], pri_r(0), pri_r(1))
                for r in range(2, R):
                    nc.vector.tensor_add(ssum[:], ssum[:], pri_r(r))

            logits = rt.tile([B, R], F32, tag="logits")
            vote = rt.tile([B, L], F32, tag="vote")

            def squash_scale(v, sqscale, tag):
                """[B,1] tile: sqrt(sq)/(1+sq), sq = sum(v*v)*sqscale."""
                sqv = rt.tile([B, 1], F32, tag=tag + "sq", name=tag + "sq")
                junk = junkp.tile([B, L], F32, tag="junk", name="junk")
                sqr = rt.tile([B, 1], F32, tag=tag + "sr", name=tag + "sr")
                nc.vector.scalar_tensor_tensor(
                    out=junk[:], in0=v, scalar=1.0, in1=v,
                    op0=ALU.mult, op1=ALU.mult, accum_out=sqr[:])
                nc.vector.tensor_scalar_mul(sqv[:], sqr[:], float(sqscale))
                a = rt.tile([B, 1], F32, tag=tag + "a", name=tag + "a")
                nc.scalar.activation(a[:], sqv[:], ACTF.Sqrt)
                bb = rt.tile([B, 1], F32, tag=tag + "b", name=tag + "b")
                nc.vector.tensor_scalar_add(bb[:], sqv[:], 1.0)
                cc = rt.tile([B, 1], F32, tag=tag + "c", name=tag + "c")
                nc.vector.reciprocal(cc[:], bb[:])
                sc = rt.tile([B, 1], F32, tag=tag + "s", name=tag + "s")
                nc.vector.tensor_mul(sc[:], a[:], cc[:])
                return sc

            def raw_delta(vsrc, dst):
                """dst[b, r] = sum_l pri_r * vsrc."""
                for r in range(R):
                    junk = junkp.tile([B, L], F32, tag="junk", name="junk")
                    nc.vector.scalar_tensor_tensor(
                        out=junk[:], in0=pri_r(r), scalar=1.0, in1=vsrc,
                        op0=ALU.mult, op1=ALU.mult,
                        accum_out=dst[:, r:r + 1])

            def softmax_and_vote(lg, vdst):
                mx = rt.tile([B, 1], F32, tag="mx", name="mx")
                nc.vector.tensor_reduce(out=mx[:], in_=lg[:], axis=AX.X,
                                        op=ALU.max)
                ngm = rt.tile([B, 1], F32, tag="ngm", name="ngm")
                nc.vector.tensor_scalar_mul(ngm[:], mx[:], -1.0)
                ex = rt.tile([B, R], F32, tag="ex", name="ex")
                nc.scalar.activation(ex[:], lg[:], ACTF.Exp,
                                     bias=ngm[0:B, 0:1])
                se = rt.tile([B, 1], F32, tag="se", name="se")
                nc.vector.tensor_reduce(out=se[:], in_=ex[:], axis=AX.X,
                                        op=ALU.add)
                ri = rt.tile([B, 1], F32, tag="ri", name="ri")
                nc.vector.reciprocal(ri[:], se[:])
                pr = rt.tile([B, R], F32, tag="pr", name="pr")
                nc.vector.tensor_scalar_mul(pr[:], ex[:], ri[0:B, 0:1])
                # vote = sum_r probs_r * pri_r
                acc = accp.tile([B, L], F32, tag="acc", name="acc")
                nc.vector.tensor_scalar_mul(acc[:], pri_r(0), pr[0:B, 0:1])
                for r in range(1, R):
                    acc2 = accp.tile([B, L], F32, tag="acc", name="acc")
                    nc.vector.scalar_tensor_tensor(
                        out=acc2[:], in0=pri_r(r), scalar=pr[0:B, r:r + 1],
                        in1=acc[:], op0=ALU.mult, op1=ALU.add)
                    acc = acc2
                nc.vector.tensor_copy(vdst, acc[:])

            # iter 0
            sc0 = squash_scale(ssum[:], 1.0 / (R * R), "i0")
            rd0 = rt.tile([B, R], F32, tag="rd0")
            raw_delta(ssum[:], rd0)
            t0 = rt.tile([B, R], F32, tag="t0")
            nc.vector.tensor_scalar_mul(t0[:], rd0[:], sc0[0:B, 0:1])
            nc.vector.tensor_scalar_mul(logits[:], t0[:], 1.0 / R)

            # iter 1
            softmax_and_vote(logits, vote[:])
            sc1 = squash_scale(vote[:], 1.0, "i1")
            rd1 = rt.tile([B, R], F32, tag="rd1")
            raw_delta(vote[:], rd1)
            t1 = rt.tile([B, R], F32, tag="t1")
            nc.vector.tensor_scalar_mul(t1[:], rd1[:], sc1[0:B, 0:1])
            lg2 = rt.tile([B, R], F32, tag="lg2")
            nc.vector.tensor_add(lg2[:], logits[:], t1[:])

            # iter 2 (final vote; reference uses the un-squashed vote)
            softmax_and_vote(lg2, vote[:])

            # ============= stage D: reinterpret + final matmul =========
            # vote cols are (c', sl) thanks to the host-side rw column
            # permutation; round trip through DRAM to regroup partitions.
            vote_b = rt.tile([B, L], BF16, tag="vote_b")
            nc.vector.tensor_copy(vote_b[:], vote[:])
            votedram = vdp.tile([B, L], BF16, tag="votedram")
            nc.sync.dma_start(out=votedram[:], in_=vote_b[:])
            # hT[c', m*256+j*32+sl] = votedram[m*8+j, c'*32+sl]
            hT = rt.tile([CAP, BLOC * S], BF16, tag="hT")
            nc.sync.dma_start(
                out=hT[:].rearrange("p (m j sl) -> p m j sl", m=BLOC, j=8),
                in_=votedram[:].rearrange("(m j) (c sl) -> c m j sl",
                                          j=8, sl=SL))

            ps_stack = ExitStack()
            psO = ps_stack.enter_context(
                tc.tile_pool(name="psO", bufs=3, space="PSUM"))
            for t in range(16):
                pso = psO.tile([128, D], F32, tag="pso")
                nc.tensor.matmul(
                    pso[:, 0:512], lhsT=hT[:, t * 128:(t + 1) * 128],
                    rhs=lwt_sb[:, 0:512], start=True, stop=True)
                nc.tensor.matmul(
                    pso[:, 512:D], lhsT=hT[:, t * 128:(t + 1) * 128],
                    rhs=lwt_sb[:, 512:D], start=True, stop=True)
                o_sb = osbp.tile([128, D], BF16, tag="osb")
                copy_alt(o_sb[:], pso[:])
                nc.sync.dma_start(out=out[t * 128:(t + 1) * 128, :],
                                  in_=o_sb[:])
            ps_stack.close()

    nc.compile()
    return nc


# stage D needs vote columns in (c', sl) order; permute rw's class dim
# on the host: new col p holds original l = (p%32)*8 + p//32
_LPERM = (np.arange(L) % SL) * CAP + np.arange(L) // SL


def _prep_inputs(x, task, fc1_w, fc1_b, route_weights, larger_w):
    R = int(task) + 1
    fw = np.ascontiguousarray(
        fc1_w.reshape(NCOL, D).T.reshape(6, 128, NCOL).transpose(1, 0, 2)
    ).reshape(128, 6 * NCOL).astype(np.float32)
    fb = np.ascontiguousarray(fc1_b.reshape(NCOL, 1)).astype(np.float32)
    lwt = np.ascontiguousarray(larger_w.T).astype(ml_dtypes.bfloat16)
    ident = np.eye(128, dtype=np.float32)
    in_maps = []
    for i in range(N_CORES):
        xt_i = np.ascontiguousarray(
            x[:, i * SL:(i + 1) * SL, :].transpose(2, 1, 0)
        ).reshape(D, SL * B).astype(np.float32)
        rw_i = np.ascontiguousarray(
            route_weights[i, :R][:, :, _LPERM]
            .reshape(R, KT, 128, L).transpose(0, 2, 1, 3)
        ).reshape(R, 128, KT * L).astype(np.float32)
        rwh_i = rw_i.astype(ml_dtypes.bfloat16)
        rwl_i = (rw_i - rwh_i.astype(np.float32)).astype(ml_dtypes.bfloat16)
        in_maps.append({"xt": xt_i, "fw": fw, "fb": fb, "rwh": rwh_i,
                        "rwl": rwl_i, "lwt": lwt, "ident": ident})
    return in_maps


def kernel(x, task, fc1_w, fc1_b, route_weights, larger_w, larger_b,
           _return_results=False):
    x = np.asarray(x, dtype=np.float32)
    fc1_w = np.asarray(fc1_w, dtype=np.float32)
    fc1_b = np.asarray(fc1_b, dtype=np.float32)
    route_weights = np.asarray(route_weights, dtype=np.float32)
    larger_w = np.asarray(larger_w, dtype=np.float32)
    larger_b = np.asarray(larger_b, dtype=np.float32)
    R = int(task) + 1

    if R not in _cache:
        _cache[R] = _build(R)
    nc = _cache[R]

    in_maps = _prep_inputs(x, task, fc1_w, fc1_b, route_weights, larger_w)
    res = bass_utils.run_bass_kernel_spmd(nc, in_maps, list(range(N_CORES)))

    full = np.empty((B, S, D), dtype=np.float32)
    for i in range(N_CORES):
        full[i * BLOC:(i + 1) * BLOC] = (
            res.results[i]["out"].astype(np.float32).reshape(BLOC, S, D))
    if np.any(larger_b):
        full = full + larger_b[None, None, :]
    if _return_results:
        return full, res
    return full
